# revision 31
# baseline (speedup 1.0000x reference)
"""Causal self-attention with RoPE — Trainium2 Bass/Tile kernel (v2).

Problem: B=2, T=2048, C=2048, H=16 heads, D=128 head dim.
    qkv = x @ w_qkv ; RoPE(q, k) ; causal softmax attention ; out = attn_out @ w_out

Sharding (8 cores): core c handles batch b = c//4 and the 4 heads
hg = c%4 (heads 4*hg .. 4*hg+3).  Each core computes
    partial_c = attn_bh(x[b]) @ w_out[rows of its heads]      (shape [T, C])
and the host all-reduces: out[b] = sum of the 4 partials of batch b.

v2 design (vs v1 serial phases):
  * Software-pipelined single pass over 512-token chunks: block b emits
    QKV(b) ⋈ attention(b-1) ⋈ out-proj(b-2), interleaved at matmul-group
    granularity so PE never starves while ScalarE runs exps.
  * Row-sums via DVE accumulation of exp tiles (bf16) + ONE ones-matmul
    per (head, q-chunk) — removes 2 of 6 matmuls per attention pair.
  * reciprocal_approx_fast (single custom-DVE op) for 1/rowsum.
  * RoPE fused with the PSUM->SBUF eviction of q/k (no separate cast).
  * x streamed once; bf16 output partials (halves out DMA).
"""

import sys

for _p in ("/opt/trn_rl_repo",):
    if _p not in sys.path:
        sys.path.insert(0, _p)

import numpy as np
import ml_dtypes

import concourse.bass as bass
import concourse.mybir as mybir
import concourse.tile as tile

BF = mybir.dt.bfloat16
FP = mybir.dt.float32

BF_NP = ml_dtypes.bfloat16

NUM_HEADS = 16
B, T_FULL, C_FULL = 2, 2048, 2048
D = 128
N_CORES = 8
HPC = 4  # heads per core

ROPE_THETA = 10000.0


def _split_multi_waits(nc):
    """This container's walrus supports only ONE sync-wait per instruction
    ("Too many sync wait commands").  Hoist all but one wait of every
    multi-wait instruction onto preceding EventSemaphore instructions
    executed by the same engine's sequencer (block order = program order per
    engine) — same semantics, codegen-legal."""
    import bass_rust

    skip = (mybir.InstEventSemaphore,)
    ctr = 0
    for fn in nc.m.functions:
        for blk in fn.blocks:
            new_insts = None
            for idx, inst in enumerate(blk.instructions):
                si = inst.sync_info
                if (
                    not isinstance(inst, skip)
                    and si is not None
                    and si.on_wait
                    and len(si.on_wait) > 1
                ):
                    if new_insts is None:
                        new_insts = list(blk.instructions[:idx])
                    # keep the first wait (the data-dep one, usually latest to
                    # resolve) on the instruction itself; hoist the rest.
                    for w in si.on_wait[1:]:
                        ev = mybir.InstEventSemaphore(
                            name=f"I-dmaw{ctr}", ins=[], outs=[]
                        )
                        ctr += 1
                        ev.sync_info = bass_rust.SyncInfo(
                            on_wait=[w], on_update=[]
                        )
                        ev.engine = inst.engine
                        new_insts.append(ev)
                    inst.sync_info = bass_rust.SyncInfo(
                        on_wait=[si.on_wait[0]], on_update=si.on_update or []
                    )
                    new_insts.append(inst)
                elif new_insts is not None:
                    new_insts.append(inst)
            if new_insts is not None:
                blk.instructions = new_insts


class Cfg:
    """Kernel geometry. Full-size by default; shrinkable for simulator tests."""

    def __init__(self, T=T_FULL, C=C_FULL, hpc=HPC):
        assert T % 512 == 0 and C % 512 == 0
        self.T = T
        self.C = C
        self.hpc = hpc
        self.scale = 1.0 / np.sqrt(D)
        self.c_tiles = C // 128      # contraction tiles for QKV
        self.t_chunks = T // 512     # token chunks (QKV + queries)
        self.t_tiles = T // 128      # token tiles (keys / out rows)
        self.n_chunks = C // 512     # output-feature chunks for out-proj


def build_attention(cfg: Cfg):
    """Build the SPMD Bass program (identical on all cores; data differs)."""
    nc = bass.Bass("TRN2", debug=False, enable_partition_id=False)
    T, C, hpc = cfg.T, cfg.C, cfg.hpc
    F = hpc * D  # per-core q (or k, or v) feature count

    xT = nc.dram_tensor("xT", [C, T], BF, kind="ExternalInput")
    # wqk pre-packed per output-feature tile: [ft, p, (cc f)] so one 2D DMA
    # fetches one ft's full [C-chunk=128, C] weight tile.
    wqk = nc.dram_tensor("wqk", [2 * hpc * 128, C], BF, kind="ExternalInput")
    wv = nc.dram_tensor("wv", [C, F], BF, kind="ExternalInput")
    wout = nc.dram_tensor("wout", [F, C], BF, kind="ExternalInput")
    cosT = nc.dram_tensor("cosT", [D, T], BF, kind="ExternalInput")
    sinT = nc.dram_tensor("sinT", [D, T], BF, kind="ExternalInput")  # sign-baked
    masks = nc.dram_tensor("masks", [128, 4 * 512], BF, kind="ExternalInput")
    ones = nc.dram_tensor("ones", [128, 128], BF, kind="ExternalInput")
    out = nc.dram_tensor("out", [T, C], BF, kind="ExternalOutput")

    Exp = mybir.ActivationFunctionType.Exp
    Ln = mybir.ActivationFunctionType.Ln

    with tile.TileContext(nc) as tc:
        with (
            tc.tile_pool(name="sb", bufs=1) as sb,
            tc.tile_pool(name="ps", bufs=1, space="PSUM") as ps,
        ):
            # ---- weights + constants (ACT hwdge ring; needed first) ----
            # Ring plan (v2 trace: a single ring caps at ~190 GB/s and the
            # 1KB-descriptor x tiles run even slower, starving the PE early):
            #   scalar ring: wqk[0:2], then block-0 x tail, then wqk[2:],
            #                then out stores
            #   gpsimd ring: cos/sin, block-0 x middle, wv, masks, wout
            #   sync ring:   x tiles (head share)
            wqkf_sb = [
                sb.tile([128, C], BF, name=f"wqkf_sb{ft}", tag=f"wqk{ft}")
                for ft in range(2 * hpc)
            ]
            nc.scalar.dma_start(
                out=wqkf_sb[0], in_=wqk[0:128, :]
            )
            # cos/sin are consumed one 512-token slice per block: load the
            # block-0 slice up front, the rest after the block-0 x tiles
            cos_sb = sb.tile([D, T], BF, name="cos_sb")
            nc.gpsimd.dma_start(out=cos_sb[:, 0:512], in_=cosT[:, 0:512])
            sin_sb = sb.tile([D, T], BF, name="sin_sb")
            nc.gpsimd.dma_start(out=sin_sb[:, 0:512], in_=sinT[:, 0:512])
            masks_sb = sb.tile([128, 4 * 512], BF, name="masks_sb")
            ones_sb = sb.tile([128, 128], BF, name="ones_sb")
            wv_sb = [
                sb.tile([128, F], BF, name=f"wv_sb{cc}", tag=f"wv{cc}")
                for cc in range(cfg.c_tiles)
            ]
            wout_sb = [
                sb.tile([128, C], BF, name=f"wout_sb{h}", tag=f"wo{h}")
                for h in range(hpc)
            ]

            def deferred_weight_loads():
                """Emitted after block-0 x loads so the first x tiles are not
                queued behind megabytes of weights on the same rings.  wqk
                streams in half-tiles to track the per-unit consumption."""
                hc = C // 2
                for ft in range(1, 2 * hpc):
                    r = slice(ft * 128, (ft + 1) * 128)
                    nc.scalar.dma_start(
                        out=wqkf_sb[ft][:, 0:hc], in_=wqk[r, 0:hc]
                    )
                    nc.scalar.dma_start(
                        out=wqkf_sb[ft][:, hc:C], in_=wqk[r, hc:C]
                    )
                for cc in range(cfg.c_tiles):
                    nc.gpsimd.dma_start(
                        out=wv_sb[cc], in_=wv[cc * 128 : (cc + 1) * 128, :]
                    )
                if T > 512:
                    nc.gpsimd.dma_start(
                        out=cos_sb[:, 512:T], in_=cosT[:, 512:T]
                    )
                    nc.gpsimd.dma_start(
                        out=sin_sb[:, 512:T], in_=sinT[:, 512:T]
                    )
                nc.gpsimd.dma_start(out=masks_sb, in_=masks[:, :])
                nc.gpsimd.dma_start(out=ones_sb, in_=ones[:, :])
                for h in range(hpc):
                    nc.gpsimd.dma_start(
                        out=wout_sb[h], in_=wout[h * 128 : (h + 1) * 128, :]
                    )

            # ---- persistent state ----
            # q/k transposed [D, T] per head (RoPE'd); v natural [T, F].
            qk_t = [
                sb.tile([D, T], BF, name=f"qk_t{ft}", tag=f"qkt{ft}")
                for ft in range(2 * hpc)
            ]
            v_sb = sb.tile([128, cfg.t_tiles, F], BF, name="v_sb")
            otn = [[None] * cfg.t_chunks for _ in range(hpc)]

            # =============== emission units ===============

            def x_loads(tci):
                """16 x tiles for chunk tci, spread across all three DMA
                rings (1KB descriptors cap a single ring well below HBM bw)."""
                tiles = []
                n6 = (cfg.c_tiles * 6) // 16
                n11 = (cfg.c_tiles * 11) // 16
                for cc in range(cfg.c_tiles):
                    x_t = sb.tile([128, 512], BF, name=f"x{cc}",
                                  tag=f"x{cc}", bufs=2)
                    eng = (nc.sync if cc < n6
                           else nc.gpsimd if cc < n11 else nc.scalar)
                    eng.dma_start(
                        out=x_t,
                        in_=xT[cc * 128 : (cc + 1) * 128,
                               tci * 512 : (tci + 1) * 512],
                    )
                    tiles.append(x_t)
                return tiles

            # accumulate contraction tiles in x-DMA arrival order (sync-ring
            # tiles land first, then gpsimd/scalar rings interleaved)
            _n6 = (cfg.c_tiles * 6) // 16
            _n11 = (cfg.c_tiles * 11) // 16
            _g, _s = list(range(_n6, _n11)), list(range(_n11, cfg.c_tiles))
            _tail = [c for p in zip(_g, _s) for c in p]
            _tail += _g[len(_s):] + _s[len(_g):]
            cc_order = list(range(_n6)) + _tail

            def qk_unit(tci, ft, x_ch):
                """One q-or-k feature tile for chunk tci + fused RoPE."""
                sl = slice(tci * 512, (tci + 1) * 512)
                psq = ps.tile([128, 512], FP, name="psq", tag="ad", bufs=2)
                for ci, cc in enumerate(cc_order):
                    nc.tensor.matmul(
                        psq,
                        lhsT=wqkf_sb[ft][:, cc * 128 : (cc + 1) * 128],
                        rhs=x_ch[cc],
                        start=(ci == 0),
                        stop=(ci == cfg.c_tiles - 1),
                    )
                # RoPE fused with PSUM eviction:
                #   qk_t[d] = psq[d]*cos[d] + psq[(d+64)%128]*sin_baked[d]
                t1 = sb.tile([128, 512], BF, name="t1", tag="rt1", bufs=2)
                nc.vector.tensor_mul(t1, psq, cos_sb[:, sl])
                t2 = sb.tile([128, 512], BF, name="t2", tag="rt2", bufs=2)
                nc.vector.tensor_mul(t2[0:64, :], psq[64:128, :], sin_sb[0:64, sl])
                nc.vector.tensor_mul(t2[64:128, :], psq[0:64, :], sin_sb[64:128, sl])
                nc.vector.tensor_add(qk_t[ft][:, sl], t1, t2)

            def v_unit(tci, tt, x_ch):
                """One 128-token v tile for chunk tci."""
                psv = ps.tile([128, F], FP, name="psv", tag="ad", bufs=2)
                for ci, cc in enumerate(cc_order):
                    nc.tensor.matmul(
                        psv,
                        lhsT=x_ch[cc][:, tt * 128 : (tt + 1) * 128],
                        rhs=wv_sb[cc],
                        start=(ci == 0),
                        stop=(ci == cfg.c_tiles - 1),
                    )
                nc.scalar.copy(v_sb[:, tci * 4 + tt, :], psv)

            osb_box = [None]

            def d_unit(qc, tt4, n):
                """Out-proj for (row tile qc*4+tt4, 512-col chunk n)."""
                tt = qc * 4 + tt4
                if n == 0:
                    osb_box[0] = sb.tile([128, C], BF, name="osb",
                                         tag="osb", bufs=2)
                osb = osb_box[0]
                pso = ps.tile([128, 512], FP, name="pso", tag="ad", bufs=2)
                for h in range(hpc):
                    nc.tensor.matmul(
                        pso,
                        lhsT=otn[h][qc][:, tt4 * 128 : (tt4 + 1) * 128],
                        rhs=wout_sb[h][:, n * 512 : (n + 1) * 512],
                        start=(h == 0),
                        stop=(h == hpc - 1),
                    )
                nc.scalar.copy(osb[:, n * 512 : (n + 1) * 512], pso)
                # per-slice DMA on the sync ring (nearly idle; the DMA-config
                # cost was eating ScalarE time) — shrinks the final drain to
                # one 128KB slice
                nc.sync.dma_start(
                    out=out[tt * 128 : (tt + 1) * 128, n * 512 : (n + 1) * 512],
                    in_=osb[:, n * 512 : (n + 1) * 512],
                )

            def b_steps(h, qc):
                """Attention for (head h, query chunk qc): generator yielding
                once per key-tile pair so the scheduler can interpose PE work
                between the scores matmul and the exp-dependent av matmul."""
                q_sl = qk_t[h][:, qc * 512 : (qc + 1) * 512]
                k_h = qk_t[hpc + h]
                nkp = 2 * (qc + 1)
                avps = ps.tile([128, 512], FP, name="avps", tag="av", bufs=2)
                esum = sb.tile([128, 1024], BF, name="esum", tag="esum", bufs=2)
                exps = {}

                # trimmed diagonal pairs (qc>=1): diagonal key-tile m only
                # needs q >= 128*m.  Layouts are q-aligned (tile m's q-slice
                # q0: lives at column 512+q0 when in the second half) so the
                # final halves-fold still produces per-q rowsums.
                #   pair 'A' (m=0,1): [0:512] full + [640:1024] = q[128:512]
                #   pair 'B' (m=2,3): [256:512] = q[256:512] + [896:1024]
                #                      = q[384:512]
                trim = qc >= 1
                tri = masks_sb[:, 0:128]  # [128,128] k<=q' triangle

                def _ranges(jp):
                    if trim and jp == nkp - 2:
                        return "A", ((0, 512, 0), (640, 1024, 128))
                    if trim and jp == nkp - 1:
                        return "B", ((256, 512, 256), (896, 1024, 384))
                    return None, ((0, 512, 0), (512, 1024, 0))

                def sc_exp(jp):
                    j0, j1 = 2 * jp, 2 * jp + 1
                    kind, rng = _ranges(jp)
                    scps = ps.tile([128, 1024], FP, name="scps",
                                   tag="sc", bufs=2)
                    for (c0, c1, q0), j in zip(rng, (j0, j1)):
                        nc.tensor.matmul(
                            scps[:, c0:c1],
                            lhsT=k_h[:, j * 128 : (j + 1) * 128],
                            rhs=q_sl[:, q0:512],
                            start=True,
                            stop=True,
                        )
                    expT = sb.tile([128, 1024], BF, name="expT",
                                   tag="exp", bufs=4)
                    if kind is None:
                        nc.scalar.activation(expT, scps, Exp,
                                             scale=float(cfg.scale))
                        if jp >= nkp - 2:  # untrimmed diagonal (qc==0)
                            mi = jp - (nkp - 2)
                            nc.vector.tensor_mul(
                                expT, expT,
                                masks_sb[:, mi * 1024 : (mi + 1) * 1024],
                            )
                    else:
                        for c0, c1, _ in rng:
                            nc.scalar.activation(
                                expT[:, c0:c1], scps[:, c0:c1], Exp,
                                scale=float(cfg.scale),
                            )
                            # triangle mask on the leading 128 cols (the
                            # tile's own diagonal block); the rest is causal
                            nc.vector.tensor_mul(
                                expT[:, c0 : c0 + 128], expT[:, c0 : c0 + 128],
                                tri,
                            )
                    exps[jp] = expT

                def av_presum(jp, first, last):
                    expT = exps.pop(jp)
                    j0, j1 = 2 * jp, 2 * jp + 1
                    kind, rng = _ranges(jp)
                    for (c0, c1, q0), j in zip(rng, (j0, j1)):
                        nc.tensor.matmul(
                            avps[:, q0:512],
                            lhsT=v_sb[:, j, h * 128 : (h + 1) * 128],
                            rhs=expT[:, c0:c1],
                            start=first and c0 == 0,
                            stop=last and c1 == 1024,
                        )
                    if kind is None:
                        if first:
                            nc.vector.tensor_copy(esum, expT)
                        else:
                            nc.vector.tensor_add(esum, esum, expT)
                    elif kind == "A":
                        nc.vector.tensor_copy(esum[:, 0:512], expT[:, 0:512])
                        nc.vector.tensor_copy(
                            esum[:, 640:1024], expT[:, 640:1024]
                        )
                        nc.vector.memset(esum[:, 512:640], 0)
                    else:
                        nc.vector.tensor_add(
                            esum[:, 256:512], esum[:, 256:512],
                            expT[:, 256:512],
                        )
                        nc.vector.tensor_add(
                            esum[:, 896:1024], esum[:, 896:1024],
                            expT[:, 896:1024],
                        )

                # diagonal pairs first: their post-exp mask multiply (DVE)
                # adds latency before the av matmul can run; fronting them
                # hides it behind the rest of the pair pipeline.
                order = [nkp - 2, nkp - 1] + list(range(nkp - 2))
                for pos, jp in enumerate(order):
                    sc_exp(jp)
                    yield
                    if pos >= 1:
                        pjp = order[pos - 1]
                        av_presum(pjp, first=(pos == 1), last=False)
                av_presum(order[-1], first=(nkp == 1), last=True)
                # epilogue: rowsums -> 1/rowsum -> normalized attn out
                nc.vector.tensor_add(
                    esum[:, 0:512], esum[:, 0:512], esum[:, 512:1024]
                )
                dnps = ps.tile([128, 1024], FP, name="dnps", tag="sc", bufs=2)
                nc.tensor.matmul(
                    dnps[:, 0:512], lhsT=ones_sb, rhs=esum[:, 0:512],
                    start=True, stop=True,
                )
                # 1/rowsum as exp(-ln(x)) on ScalarE: ln+exp share one act
                # table (natural_log_exp_and_others) with the scores exp,
                # so no table reloads; DVE reciprocal is 6x slower.
                lnd = sb.tile([128, 512], FP, name="lnd", tag="lnd", bufs=2)
                nc.scalar.activation(lnd, dnps[:, 0:512], Ln)
                rsrec = sb.tile([128, 512], FP, name="rsrec",
                                tag="rsrec", bufs=2)
                nc.scalar.activation(rsrec, lnd, Exp, scale=-1.0)
                o = sb.tile([128, 512], BF, name=f"otn{h}_{qc}",
                            tag=f"otn{h}", bufs=4)
                nc.vector.tensor_mul(o, avps, rsrec)
                otn[h][qc] = o

            # =============== scheduler ===============

            def chain_b(qc):
                for h in range(hpc):
                    yield from b_steps(h, qc)

            def interleave_even(a, b):
                """Merge two unit lists evenly (Bresenham)."""
                if not a:
                    return list(b)
                if not b:
                    return list(a)
                res, ai, bi = [], 0, 0
                na, nb = len(a), len(b)
                while ai < na or bi < nb:
                    if bi >= nb or (ai < na and ai * nb <= bi * na):
                        res.append(a[ai]); ai += 1
                    else:
                        res.append(b[bi]); bi += 1
                return res

            def merge(units, bgen, n_bsteps):
                """Interleave unit closures with pulls from the B generator,
                spread evenly by count."""
                if bgen is None:
                    for u in units:
                        u()
                    return
                # hold back a unit or three to emit after the generator
                # drains: they keep the PE fed while the last (h,qc) epilogue
                # chain (fold -> ones -> ln -> exp -> mul) resolves on
                # Act/DVE.  Only small D units are held back aggressively;
                # big A units must stay spread through the B pairs.
                hold = 1 if b < cfg.t_chunks else min(3, len(units))
                nu = max(0, len(units) - hold)
                ui = 0
                for bi in range(n_bsteps):
                    # emit units scheduled before this b-step
                    while ui < nu and ui * n_bsteps < bi * nu:
                        units[ui](); ui += 1
                    try:
                        next(bgen)
                    except StopIteration:
                        break
                # drain
                for step in bgen:
                    pass
                while ui < len(units):
                    units[ui](); ui += 1

            n_blocks = cfg.t_chunks + 2
            for b in range(n_blocks):
                units = []
                if b < cfg.t_chunks:
                    x_ch = x_loads(b)
                    if b == 0:
                        deferred_weight_loads()
                    a_units = [
                        (lambda tci=b, ft=ft, xc=x_ch: qk_unit(tci, ft, xc))
                        for ft in range(2 * hpc)
                    ] + [
                        (lambda tci=b, tt=tt, xc=x_ch: v_unit(tci, tt, xc))
                        for tt in range(4)
                    ]
                    units = a_units
                if b >= 2:
                    qc = b - 2
                    dunits = [
                        (lambda qc=qc, tt4=tt4, n=n: d_unit(qc, tt4, n))
                        for tt4 in range(4)
                        for n in range(cfg.n_chunks)
                    ]
                    units = interleave_even(units, dunits)
                if 1 <= b <= cfg.t_chunks:
                    qc = b - 1
                    merge(units, chain_b(qc), hpc * 2 * (qc + 1))
                else:
                    merge(units, None, 0)

    return nc


def rope_tables(T, dtype=np.float32):
    inv_freq = 1.0 / (ROPE_THETA ** (np.arange(0, D, 2, dtype=np.float32) / D))
    t = np.arange(T, dtype=np.float32)
    freqs = np.outer(t, inv_freq)  # [T, D/2]
    emb = np.concatenate([freqs, freqs], axis=-1)  # [T, D]
    return np.cos(emb).astype(dtype), np.sin(emb).astype(dtype)


def make_core_inputs(cfg: Cfg, x_b, w_qkv, w_out, cos, sin, hg):
    """Per-core input dict. x_b [T, C] fp32; w_qkv [C, 3C']; w_out [C', C];
    cos/sin [T, D]; hg = head-group index within the batch group."""
    T, C, hpc = cfg.T, cfg.C, cfg.hpc
    F = hpc * D
    H = w_qkv.shape[1] // 3 // D  # total heads in this (possibly shrunk) problem
    CQ = H * D

    f0 = hg * F
    xT = np.ascontiguousarray(x_b.T).astype(BF_NP)
    wq = w_qkv[:, f0 : f0 + F]
    wk = w_qkv[:, CQ + f0 : CQ + f0 + F]
    W = np.concatenate([wq, wk], axis=1)  # [C, 2F]
    # pack per-ft: wqk[ft*128+p, cc*128+f] = W[cc*128+p, ft*128+f]
    nft, ncc = 2 * hpc, C // 128
    wqk = np.ascontiguousarray(
        W.reshape(ncc, 128, nft, 128).transpose(2, 1, 0, 3).reshape(
            nft * 128, ncc * 128
        )
    ).astype(BF_NP)
    wv = np.ascontiguousarray(w_qkv[:, 2 * CQ + f0 : 2 * CQ + f0 + F]).astype(BF_NP)
    wout = np.ascontiguousarray(w_out[f0 : f0 + F, :]).astype(BF_NP)

    cosT = np.ascontiguousarray(cos.T).astype(BF_NP)  # [D, T]
    sinT = np.ascontiguousarray(sin.T).astype(np.float32)
    sinT[0:64, :] *= -1.0  # bake rotate_half sign
    sinT = sinT.astype(BF_NP)

    # diagonal-group masks: mask[mi][k, q] = 1 iff mi*128 + k <= q
    k_idx = np.arange(128)[:, None]
    q_idx = np.arange(512)[None, :]
    m = np.concatenate(
        [(mi * 128 + k_idx <= q_idx) for mi in range(4)], axis=1
    ).astype(BF_NP)

    return {
        "xT": xT,
        "wqk": wqk,
        "wv": wv,
        "wout": wout,
        "cosT": cosT,
        "sinT": sinT,
        "masks": np.ascontiguousarray(m),
        "ones": np.ones((128, 128), dtype=BF_NP),
    }


_NC_CACHE = {}


def _get_nc(cfg: Cfg):
    key = (cfg.T, cfg.C, cfg.hpc)
    if key not in _NC_CACHE:
        nc = build_attention(cfg)
        _split_multi_waits(nc)  # HW codegen needs ≤1 wait per instruction
        _NC_CACHE[key] = nc
    return _NC_CACHE[key]


def kernel(x, cos, sin, w_qkv, w_out, trace=False, tmpdir=None):
    """Full-problem entry point: full inputs in, full [B, T, C] output back."""
    from concourse.bass_utils import run_bass_kernel_spmd

    x = np.asarray(x, dtype=np.float32)
    cos = np.asarray(cos, dtype=np.float32)
    sin = np.asarray(sin, dtype=np.float32)
    w_qkv = np.asarray(w_qkv, dtype=np.float32)
    w_out = np.asarray(w_out, dtype=np.float32)

    cfg = Cfg()
    nc = _get_nc(cfg)

    in_maps = []
    for c in range(N_CORES):
        b, hg = c // 4, c % 4
        in_maps.append(
            make_core_inputs(cfg, x[b], w_qkv, w_out, cos, sin, hg)
        )

    res = run_bass_kernel_spmd(
        nc,
        in_maps,
        core_ids=list(range(N_CORES)),
        trace=trace,
        tmpdir=tmpdir,
    )
    partials = [np.asarray(r["out"], dtype=np.float32) for r in res.results]
    out = np.empty((B, cfg.T, cfg.C), dtype=np.float32)
    for b in range(B):
        out[b] = partials[4 * b] + partials[4 * b + 1]
        out[b] += partials[4 * b + 2]
        out[b] += partials[4 * b + 3]
    if trace:
        return out, res
    return out


# revision 33
# speedup vs baseline: 1.0423x; 1.0423x over previous
"""Causal self-attention with RoPE — Trainium2 Bass/Tile kernel (v2).

Problem: B=2, T=2048, C=2048, H=16 heads, D=128 head dim.
    qkv = x @ w_qkv ; RoPE(q, k) ; causal softmax attention ; out = attn_out @ w_out

Sharding (8 cores): core c handles batch b = c//4 and the 4 heads
hg = c%4 (heads 4*hg .. 4*hg+3).  Each core computes
    partial_c = attn_bh(x[b]) @ w_out[rows of its heads]      (shape [T, C])
and the host all-reduces: out[b] = sum of the 4 partials of batch b.

v2 design (vs v1 serial phases):
  * Software-pipelined single pass over 512-token chunks: block b emits
    QKV(b) ⋈ attention(b-1) ⋈ out-proj(b-2), interleaved at matmul-group
    granularity so PE never starves while ScalarE runs exps.
  * Row-sums via DVE accumulation of exp tiles (bf16) + ONE ones-matmul
    per (head, q-chunk) — removes 2 of 6 matmuls per attention pair.
  * reciprocal_approx_fast (single custom-DVE op) for 1/rowsum.
  * RoPE fused with the PSUM->SBUF eviction of q/k (no separate cast).
  * x streamed once; bf16 output partials (halves out DMA).
"""

import sys

for _p in ("/opt/trn_rl_repo",):
    if _p not in sys.path:
        sys.path.insert(0, _p)

import numpy as np
import ml_dtypes

import concourse.bass as bass
import concourse.mybir as mybir
import concourse.tile as tile

BF = mybir.dt.bfloat16
FP = mybir.dt.float32

BF_NP = ml_dtypes.bfloat16

NUM_HEADS = 16
B, T_FULL, C_FULL = 2, 2048, 2048
D = 128
N_CORES = 8
HPC = 4  # heads per core

ROPE_THETA = 10000.0


def _split_multi_waits(nc):
    """This container's walrus supports only ONE sync-wait per instruction
    ("Too many sync wait commands").  Hoist all but one wait of every
    multi-wait instruction onto preceding EventSemaphore instructions
    executed by the same engine's sequencer (block order = program order per
    engine) — same semantics, codegen-legal."""
    import bass_rust

    skip = (mybir.InstEventSemaphore,)
    ctr = 0
    for fn in nc.m.functions:
        for blk in fn.blocks:
            new_insts = None
            for idx, inst in enumerate(blk.instructions):
                si = inst.sync_info
                if (
                    not isinstance(inst, skip)
                    and si is not None
                    and si.on_wait
                    and len(si.on_wait) > 1
                ):
                    if new_insts is None:
                        new_insts = list(blk.instructions[:idx])
                    # keep the first wait (the data-dep one, usually latest to
                    # resolve) on the instruction itself; hoist the rest.
                    for w in si.on_wait[1:]:
                        ev = mybir.InstEventSemaphore(
                            name=f"I-dmaw{ctr}", ins=[], outs=[]
                        )
                        ctr += 1
                        ev.sync_info = bass_rust.SyncInfo(
                            on_wait=[w], on_update=[]
                        )
                        ev.engine = inst.engine
                        new_insts.append(ev)
                    inst.sync_info = bass_rust.SyncInfo(
                        on_wait=[si.on_wait[0]], on_update=si.on_update or []
                    )
                    new_insts.append(inst)
                elif new_insts is not None:
                    new_insts.append(inst)
            if new_insts is not None:
                blk.instructions = new_insts


class Cfg:
    """Kernel geometry. Full-size by default; shrinkable for simulator tests."""

    def __init__(self, T=T_FULL, C=C_FULL, hpc=HPC):
        assert T % 512 == 0 and C % 512 == 0
        self.T = T
        self.C = C
        self.hpc = hpc
        self.scale = 1.0 / np.sqrt(D)
        self.c_tiles = C // 128      # contraction tiles for QKV
        self.t_chunks = T // 512     # token chunks (QKV + queries)
        self.t_tiles = T // 128      # token tiles (keys / out rows)
        self.n_chunks = C // 512     # output-feature chunks for out-proj


def build_attention(cfg: Cfg):
    """Build the SPMD Bass program (identical on all cores; data differs)."""
    nc = bass.Bass("TRN2", debug=False, enable_partition_id=False)
    T, C, hpc = cfg.T, cfg.C, cfg.hpc
    F = hpc * D  # per-core q (or k, or v) feature count

    xT = nc.dram_tensor("xT", [C, T], BF, kind="ExternalInput")
    # wqk pre-packed per output-feature tile: [ft, p, (cc f)] so one 2D DMA
    # fetches one ft's full [C-chunk=128, C] weight tile.
    wqk = nc.dram_tensor("wqk", [2 * hpc * 128, C], BF, kind="ExternalInput")
    wv = nc.dram_tensor("wv", [C, F], BF, kind="ExternalInput")
    wout = nc.dram_tensor("wout", [F, C], BF, kind="ExternalInput")
    cosT = nc.dram_tensor("cosT", [D, T], BF, kind="ExternalInput")
    sinT = nc.dram_tensor("sinT", [D, T], BF, kind="ExternalInput")  # sign-baked
    masks = nc.dram_tensor("masks", [128, 4 * 512], BF, kind="ExternalInput")
    ones = nc.dram_tensor("ones", [128, 128], BF, kind="ExternalInput")
    out = nc.dram_tensor("out", [T, C], BF, kind="ExternalOutput")

    Exp = mybir.ActivationFunctionType.Exp
    Ln = mybir.ActivationFunctionType.Ln

    with tile.TileContext(nc) as tc:
        with (
            tc.tile_pool(name="sb", bufs=1) as sb,
            tc.tile_pool(name="ps", bufs=1, space="PSUM") as ps,
        ):
            # ---- weights + constants (ACT hwdge ring; needed first) ----
            # Ring plan (v2 trace: a single ring caps at ~190 GB/s and the
            # 1KB-descriptor x tiles run even slower, starving the PE early):
            #   scalar ring: wqk[0:2], then block-0 x tail, then wqk[2:],
            #                then out stores
            #   gpsimd ring: cos/sin, block-0 x middle, wv, masks, wout
            #   sync ring:   x tiles (head share)
            wqkf_sb = [
                sb.tile([128, C], BF, name=f"wqkf_sb{ft}", tag=f"wqk{ft}")
                for ft in range(2 * hpc)
            ]
            nc.scalar.dma_start(
                out=wqkf_sb[0], in_=wqk[0:128, :]
            )
            # cos/sin are consumed one 512-token slice per block: load the
            # block-0 slice up front, the rest after the block-0 x tiles
            cos_sb = sb.tile([D, T], BF, name="cos_sb")
            nc.gpsimd.dma_start(out=cos_sb[:, 0:512], in_=cosT[:, 0:512])
            sin_sb = sb.tile([D, T], BF, name="sin_sb")
            nc.gpsimd.dma_start(out=sin_sb[:, 0:512], in_=sinT[:, 0:512])
            masks_sb = sb.tile([128, 4 * 512], BF, name="masks_sb")
            ones_sb = sb.tile([128, 128], BF, name="ones_sb")
            wv_sb = [
                sb.tile([128, F], BF, name=f"wv_sb{cc}", tag=f"wv{cc}")
                for cc in range(cfg.c_tiles)
            ]
            wout_sb = [
                sb.tile([128, C], BF, name=f"wout_sb{h}", tag=f"wo{h}")
                for h in range(hpc)
            ]

            def deferred_weight_loads():
                """Emitted after block-0 x loads so the first x tiles are not
                queued behind megabytes of weights on the same rings.  wqk
                streams in half-tiles to track the per-unit consumption."""
                hc = C // 2
                for ft in range(1, 2 * hpc):
                    r = slice(ft * 128, (ft + 1) * 128)
                    nc.scalar.dma_start(
                        out=wqkf_sb[ft][:, 0:hc], in_=wqk[r, 0:hc]
                    )
                    nc.scalar.dma_start(
                        out=wqkf_sb[ft][:, hc:C], in_=wqk[r, hc:C]
                    )
                for cc in range(cfg.c_tiles):
                    nc.gpsimd.dma_start(
                        out=wv_sb[cc], in_=wv[cc * 128 : (cc + 1) * 128, :]
                    )
                if T > 512:
                    nc.gpsimd.dma_start(
                        out=cos_sb[:, 512:T], in_=cosT[:, 512:T]
                    )
                    nc.gpsimd.dma_start(
                        out=sin_sb[:, 512:T], in_=sinT[:, 512:T]
                    )
                nc.gpsimd.dma_start(out=masks_sb, in_=masks[:, :])
                nc.gpsimd.dma_start(out=ones_sb, in_=ones[:, :])
                for h in range(hpc):
                    nc.gpsimd.dma_start(
                        out=wout_sb[h], in_=wout[h * 128 : (h + 1) * 128, :]
                    )

            # ---- persistent state ----
            # q/k transposed [D, T] per head (RoPE'd); v natural [T, F].
            qk_t = [
                sb.tile([D, T], BF, name=f"qk_t{ft}", tag=f"qkt{ft}")
                for ft in range(2 * hpc)
            ]
            v_sb = sb.tile([128, cfg.t_tiles, F], BF, name="v_sb")
            otn = [[None] * cfg.t_chunks for _ in range(hpc)]

            # =============== emission units ===============

            def x_loads(tci):
                """16 x tiles for chunk tci, spread across all three DMA
                rings (1KB descriptors cap a single ring well below HBM bw)."""
                tiles = []
                n6 = (cfg.c_tiles * 6) // 16
                n11 = (cfg.c_tiles * 11) // 16
                for cc in range(cfg.c_tiles):
                    x_t = sb.tile([128, 512], BF, name=f"x{cc}",
                                  tag=f"x{cc}", bufs=2)
                    eng = (nc.sync if cc < n6
                           else nc.gpsimd if cc < n11 else nc.scalar)
                    eng.dma_start(
                        out=x_t,
                        in_=xT[cc * 128 : (cc + 1) * 128,
                               tci * 512 : (tci + 1) * 512],
                    )
                    tiles.append(x_t)
                return tiles

            # accumulate contraction tiles in x-DMA arrival order (sync-ring
            # tiles land first, then gpsimd/scalar rings interleaved)
            _n6 = (cfg.c_tiles * 6) // 16
            _n11 = (cfg.c_tiles * 11) // 16
            _g, _s = list(range(_n6, _n11)), list(range(_n11, cfg.c_tiles))
            _tail = [c for p in zip(_g, _s) for c in p]
            _tail += _g[len(_s):] + _s[len(_g):]
            cc_order = list(range(_n6)) + _tail

            def qk_unit(tci, ft, x_ch):
                """One q-or-k feature tile for chunk tci + fused RoPE."""
                sl = slice(tci * 512, (tci + 1) * 512)
                psq = ps.tile([128, 512], FP, name="psq", tag="ad", bufs=2)
                for ci, cc in enumerate(cc_order):
                    nc.tensor.matmul(
                        psq,
                        lhsT=wqkf_sb[ft][:, cc * 128 : (cc + 1) * 128],
                        rhs=x_ch[cc],
                        start=(ci == 0),
                        stop=(ci == cfg.c_tiles - 1),
                    )
                # RoPE fused with PSUM eviction:
                #   qk_t[d] = psq[d]*cos[d] + psq[(d+64)%128]*sin_baked[d]
                t1 = sb.tile([128, 512], BF, name="t1", tag="rt1", bufs=2)
                nc.vector.tensor_mul(t1, psq, cos_sb[:, sl])
                t2 = sb.tile([128, 512], BF, name="t2", tag="rt2", bufs=2)
                nc.vector.tensor_mul(t2[0:64, :], psq[64:128, :], sin_sb[0:64, sl])
                nc.vector.tensor_mul(t2[64:128, :], psq[0:64, :], sin_sb[64:128, sl])
                nc.vector.tensor_add(qk_t[ft][:, sl], t1, t2)

            def v_unit(tci, tt, x_ch):
                """One 128-token v tile for chunk tci."""
                psv = ps.tile([128, F], FP, name="psv", tag="ad", bufs=2)
                for ci, cc in enumerate(cc_order):
                    nc.tensor.matmul(
                        psv,
                        lhsT=x_ch[cc][:, tt * 128 : (tt + 1) * 128],
                        rhs=wv_sb[cc],
                        start=(ci == 0),
                        stop=(ci == cfg.c_tiles - 1),
                    )
                nc.scalar.copy(v_sb[:, tci * 4 + tt, :], psv)

            osb_box = [None]

            def d_unit(qc, tt4, n, alt=False):
                """Out-proj for (row tile qc*4+tt4, 512-col chunk n)."""
                tt = qc * 4 + tt4
                if n == 0:
                    osb_box[0] = sb.tile([128, C], BF, name="osb",
                                         tag="osb", bufs=2)
                osb = osb_box[0]
                pso = ps.tile([128, 512], FP, name="pso", tag="ad", bufs=2)
                for h in range(hpc):
                    nc.tensor.matmul(
                        pso,
                        lhsT=otn[h][qc][:, tt4 * 128 : (tt4 + 1) * 128],
                        rhs=wout_sb[h][:, n * 512 : (n + 1) * 512],
                        start=(h == 0),
                        stop=(h == hpc - 1),
                    )
                # in tail blocks ScalarE is exp-saturated; alternating the
                # PSUM eviction with DVE keeps the psum rotation feeding PE
                if alt and (tt4 + n) % 2 == 1:
                    nc.vector.tensor_copy(osb[:, n * 512 : (n + 1) * 512], pso)
                else:
                    nc.scalar.copy(osb[:, n * 512 : (n + 1) * 512], pso)
                # per-slice DMA on the sync ring (nearly idle; the DMA-config
                # cost was eating ScalarE time) — shrinks the final drain to
                # one 128KB slice
                nc.sync.dma_start(
                    out=out[tt * 128 : (tt + 1) * 128, n * 512 : (n + 1) * 512],
                    in_=osb[:, n * 512 : (n + 1) * 512],
                )

            def b_steps(h, qc):
                """Attention for (head h, query chunk qc): generator yielding
                once per key-tile pair so the scheduler can interpose PE work
                between the scores matmul and the exp-dependent av matmul."""
                q_sl = qk_t[h][:, qc * 512 : (qc + 1) * 512]
                k_h = qk_t[hpc + h]
                nkp = 2 * (qc + 1)
                avps = ps.tile([128, 512], FP, name="avps", tag="av", bufs=2)
                esum = sb.tile([128, 1024], BF, name="esum", tag="esum", bufs=2)
                exps = {}

                # trimmed diagonal pairs (qc>=1): diagonal key-tile m only
                # needs q >= 128*m.  Layouts are q-aligned (tile m's q-slice
                # q0: lives at column 512+q0 when in the second half) so the
                # final halves-fold still produces per-q rowsums.
                #   pair 'A' (m=0,1): [0:512] full + [640:1024] = q[128:512]
                #   pair 'B' (m=2,3): [256:512] = q[256:512] + [896:1024]
                #                      = q[384:512]
                trim = qc >= 1
                tri = masks_sb[:, 0:128]  # [128,128] k<=q' triangle

                def _ranges(jp):
                    if trim and jp == nkp - 2:
                        return "A", ((0, 512, 0), (640, 1024, 128))
                    if trim and jp == nkp - 1:
                        return "B", ((256, 512, 256), (896, 1024, 384))
                    return None, ((0, 512, 0), (512, 1024, 0))

                def sc_exp(jp):
                    j0, j1 = 2 * jp, 2 * jp + 1
                    kind, rng = _ranges(jp)
                    scps = ps.tile([128, 1024], FP, name="scps",
                                   tag="sc", bufs=2)
                    for (c0, c1, q0), j in zip(rng, (j0, j1)):
                        nc.tensor.matmul(
                            scps[:, c0:c1],
                            lhsT=k_h[:, j * 128 : (j + 1) * 128],
                            rhs=q_sl[:, q0:512],
                            start=True,
                            stop=True,
                        )
                    expT = sb.tile([128, 1024], BF, name="expT",
                                   tag="exp", bufs=4)
                    if kind is None:
                        nc.scalar.activation(expT, scps, Exp,
                                             scale=float(cfg.scale))
                        if jp >= nkp - 2:  # untrimmed diagonal (qc==0)
                            mi = jp - (nkp - 2)
                            nc.vector.tensor_mul(
                                expT, expT,
                                masks_sb[:, mi * 1024 : (mi + 1) * 1024],
                            )
                    else:
                        for c0, c1, _ in rng:
                            nc.scalar.activation(
                                expT[:, c0:c1], scps[:, c0:c1], Exp,
                                scale=float(cfg.scale),
                            )
                            # triangle mask on the leading 128 cols (the
                            # tile's own diagonal block); the rest is causal
                            nc.vector.tensor_mul(
                                expT[:, c0 : c0 + 128], expT[:, c0 : c0 + 128],
                                tri,
                            )
                    exps[jp] = expT

                def av_presum(jp, first, last):
                    expT = exps.pop(jp)
                    j0, j1 = 2 * jp, 2 * jp + 1
                    kind, rng = _ranges(jp)
                    for (c0, c1, q0), j in zip(rng, (j0, j1)):
                        nc.tensor.matmul(
                            avps[:, q0:512],
                            lhsT=v_sb[:, j, h * 128 : (h + 1) * 128],
                            rhs=expT[:, c0:c1],
                            start=first and c0 == 0,
                            stop=last and c1 == 1024,
                        )
                    if kind is None:
                        if first:
                            nc.vector.tensor_copy(esum, expT)
                        else:
                            nc.vector.tensor_add(esum, esum, expT)
                    elif kind == "A":
                        nc.vector.tensor_copy(esum[:, 0:512], expT[:, 0:512])
                        nc.vector.tensor_copy(
                            esum[:, 640:1024], expT[:, 640:1024]
                        )
                        nc.vector.memset(esum[:, 512:640], 0)
                    else:
                        nc.vector.tensor_add(
                            esum[:, 256:512], esum[:, 256:512],
                            expT[:, 256:512],
                        )
                        nc.vector.tensor_add(
                            esum[:, 896:1024], esum[:, 896:1024],
                            expT[:, 896:1024],
                        )

                # diagonal pairs first: their post-exp mask multiply (DVE)
                # adds latency before the av matmul can run; fronting them
                # hides it behind the rest of the pair pipeline.
                order = [nkp - 2, nkp - 1] + list(range(nkp - 2))
                for pos, jp in enumerate(order):
                    sc_exp(jp)
                    yield
                    if pos >= 1:
                        pjp = order[pos - 1]
                        av_presum(pjp, first=(pos == 1), last=False)
                av_presum(order[-1], first=(nkp == 1), last=True)
                # epilogue: rowsums -> 1/rowsum -> normalized attn out
                nc.vector.tensor_add(
                    esum[:, 0:512], esum[:, 0:512], esum[:, 512:1024]
                )
                dnps = ps.tile([128, 1024], FP, name="dnps", tag="sc", bufs=2)
                nc.tensor.matmul(
                    dnps[:, 0:512], lhsT=ones_sb, rhs=esum[:, 0:512],
                    start=True, stop=True,
                )
                # 1/rowsum as exp(-ln(x)) on ScalarE: ln+exp share one act
                # table (natural_log_exp_and_others) with the scores exp,
                # so no table reloads; DVE reciprocal is 6x slower.
                lnd = sb.tile([128, 512], FP, name="lnd", tag="lnd", bufs=2)
                nc.scalar.activation(lnd, dnps[:, 0:512], Ln)
                rsrec = sb.tile([128, 512], FP, name="rsrec",
                                tag="rsrec", bufs=2)
                nc.scalar.activation(rsrec, lnd, Exp, scale=-1.0)
                o = sb.tile([128, 512], BF, name=f"otn{h}_{qc}",
                            tag=f"otn{h}", bufs=4)
                nc.vector.tensor_mul(o, avps, rsrec)
                otn[h][qc] = o

            # =============== scheduler ===============

            def chain_b(qc):
                for h in range(hpc):
                    yield from b_steps(h, qc)

            def interleave_even(a, b):
                """Merge two unit lists evenly (Bresenham)."""
                if not a:
                    return list(b)
                if not b:
                    return list(a)
                res, ai, bi = [], 0, 0
                na, nb = len(a), len(b)
                while ai < na or bi < nb:
                    if bi >= nb or (ai < na and ai * nb <= bi * na):
                        res.append(a[ai]); ai += 1
                    else:
                        res.append(b[bi]); bi += 1
                return res

            def merge(units, bgen, n_bsteps):
                """Interleave unit closures with pulls from the B generator,
                spread evenly by count."""
                if bgen is None:
                    for u in units:
                        u()
                    return
                # hold back a unit or three to emit after the generator
                # drains: they keep the PE fed while the last (h,qc) epilogue
                # chain (fold -> ones -> ln -> exp -> mul) resolves on
                # Act/DVE.  Only small D units are held back aggressively;
                # big A units must stay spread through the B pairs.
                hold = 1 if b < cfg.t_chunks else min(3, len(units))
                nu = max(0, len(units) - hold)
                ui = 0
                for bi in range(n_bsteps):
                    # emit units scheduled before this b-step
                    while ui < nu and ui * n_bsteps < bi * nu:
                        units[ui](); ui += 1
                    try:
                        next(bgen)
                    except StopIteration:
                        break
                # drain
                for step in bgen:
                    pass
                while ui < len(units):
                    units[ui](); ui += 1

            n_blocks = cfg.t_chunks + 2
            for b in range(n_blocks):
                units = []
                if b < cfg.t_chunks:
                    x_ch = x_loads(b)
                    if b == 0:
                        deferred_weight_loads()
                    a_units = [
                        (lambda tci=b, ft=ft, xc=x_ch: qk_unit(tci, ft, xc))
                        for ft in range(2 * hpc)
                    ] + [
                        (lambda tci=b, tt=tt, xc=x_ch: v_unit(tci, tt, xc))
                        for tt in range(4)
                    ]
                    units = a_units
                # D(qc) is emitted half in block qc+2, half in block qc+3
                # (when it exists): the tail block B(3) is Act-bound, and
                # spare D units both feed the PE there and cover the final
                # epilogue chain in the last block.
                alt = b >= cfg.t_chunks
                dunits = []
                for qc, part in ((b - 3, 1), (b - 2, 0)):
                    if not (0 <= qc < cfg.t_chunks):
                        continue
                    split = qc + 3 <= n_blocks - 1
                    if part == 0:
                        rng = range(0, 2 if split else 4)
                    elif split:
                        rng = range(2, 4)
                    else:
                        continue
                    dunits += [
                        (lambda qc=qc, tt4=tt4, n=n, alt=alt:
                         d_unit(qc, tt4, n, alt))
                        for tt4 in rng
                        for n in range(cfg.n_chunks)
                    ]
                if dunits:
                    units = interleave_even(units, dunits)
                if 1 <= b <= cfg.t_chunks:
                    qc = b - 1
                    merge(units, chain_b(qc), hpc * 2 * (qc + 1))
                else:
                    merge(units, None, 0)

    return nc


def rope_tables(T, dtype=np.float32):
    inv_freq = 1.0 / (ROPE_THETA ** (np.arange(0, D, 2, dtype=np.float32) / D))
    t = np.arange(T, dtype=np.float32)
    freqs = np.outer(t, inv_freq)  # [T, D/2]
    emb = np.concatenate([freqs, freqs], axis=-1)  # [T, D]
    return np.cos(emb).astype(dtype), np.sin(emb).astype(dtype)


def make_core_inputs(cfg: Cfg, x_b, w_qkv, w_out, cos, sin, hg):
    """Per-core input dict. x_b [T, C] fp32; w_qkv [C, 3C']; w_out [C', C];
    cos/sin [T, D]; hg = head-group index within the batch group."""
    T, C, hpc = cfg.T, cfg.C, cfg.hpc
    F = hpc * D
    H = w_qkv.shape[1] // 3 // D  # total heads in this (possibly shrunk) problem
    CQ = H * D

    f0 = hg * F
    xT = np.ascontiguousarray(x_b.T).astype(BF_NP)
    wq = w_qkv[:, f0 : f0 + F]
    wk = w_qkv[:, CQ + f0 : CQ + f0 + F]
    W = np.concatenate([wq, wk], axis=1)  # [C, 2F]
    # pack per-ft: wqk[ft*128+p, cc*128+f] = W[cc*128+p, ft*128+f]
    nft, ncc = 2 * hpc, C // 128
    wqk = np.ascontiguousarray(
        W.reshape(ncc, 128, nft, 128).transpose(2, 1, 0, 3).reshape(
            nft * 128, ncc * 128
        )
    ).astype(BF_NP)
    wv = np.ascontiguousarray(w_qkv[:, 2 * CQ + f0 : 2 * CQ + f0 + F]).astype(BF_NP)
    wout = np.ascontiguousarray(w_out[f0 : f0 + F, :]).astype(BF_NP)

    cosT = np.ascontiguousarray(cos.T).astype(BF_NP)  # [D, T]
    sinT = np.ascontiguousarray(sin.T).astype(np.float32)
    sinT[0:64, :] *= -1.0  # bake rotate_half sign
    sinT = sinT.astype(BF_NP)

    # diagonal-group masks: mask[mi][k, q] = 1 iff mi*128 + k <= q
    k_idx = np.arange(128)[:, None]
    q_idx = np.arange(512)[None, :]
    m = np.concatenate(
        [(mi * 128 + k_idx <= q_idx) for mi in range(4)], axis=1
    ).astype(BF_NP)

    return {
        "xT": xT,
        "wqk": wqk,
        "wv": wv,
        "wout": wout,
        "cosT": cosT,
        "sinT": sinT,
        "masks": np.ascontiguousarray(m),
        "ones": np.ones((128, 128), dtype=BF_NP),
    }


_NC_CACHE = {}


def _get_nc(cfg: Cfg):
    key = (cfg.T, cfg.C, cfg.hpc)
    if key not in _NC_CACHE:
        nc = build_attention(cfg)
        _split_multi_waits(nc)  # HW codegen needs ≤1 wait per instruction
        _NC_CACHE[key] = nc
    return _NC_CACHE[key]


def kernel(x, cos, sin, w_qkv, w_out, trace=False, tmpdir=None):
    """Full-problem entry point: full inputs in, full [B, T, C] output back."""
    from concourse.bass_utils import run_bass_kernel_spmd

    x = np.asarray(x, dtype=np.float32)
    cos = np.asarray(cos, dtype=np.float32)
    sin = np.asarray(sin, dtype=np.float32)
    w_qkv = np.asarray(w_qkv, dtype=np.float32)
    w_out = np.asarray(w_out, dtype=np.float32)

    cfg = Cfg()
    nc = _get_nc(cfg)

    in_maps = []
    for c in range(N_CORES):
        b, hg = c // 4, c % 4
        in_maps.append(
            make_core_inputs(cfg, x[b], w_qkv, w_out, cos, sin, hg)
        )

    res = run_bass_kernel_spmd(
        nc,
        in_maps,
        core_ids=list(range(N_CORES)),
        trace=trace,
        tmpdir=tmpdir,
    )
    partials = [np.asarray(r["out"], dtype=np.float32) for r in res.results]
    out = np.empty((B, cfg.T, cfg.C), dtype=np.float32)
    for b in range(B):
        out[b] = partials[4 * b] + partials[4 * b + 1]
        out[b] += partials[4 * b + 2]
        out[b] += partials[4 * b + 3]
    if trace:
        return out, res
    return out
